# revision 1
# baseline (speedup 1.0000x reference)
"""CosineAttention on 8 TRN2 NeuronCores.

Sharding (head + tensor parallel, per the hint):
  core c owns head h=c for both batches:
    - computes qT,kT = [Wq_h|Wk_h]^T-stationary matmuls over full xT
    - RMS-normalizes q,k in the [d, i] layout via a PE ones-matmul
      partition-reduction + K=2 outer-product broadcast
    - simT[j,i] = kn^T qn (K=64, float32r), exp folded scale=1/8 on ACT
    - attn@v with a ones column appended to v so the softmax denominator
      falls out as matmul row 64; normalize by its reciprocal
    - per-batch AllGather of o_cT [64, n] (bf16) -> [512, n] feature-major
    - column-parallel out-proj: outT_c [64, n] = W2_c^T-stationary matmul
  host concatenates the 8 feature slices.

Matmul dtype: float32r (TF32-class single-pass PE mode) for the fp32 path;
bf16 for the post-softmax path (attn weights / v / out-proj operands).
"""

import numpy as np
import ml_dtypes

import concourse.bass as bass
import concourse.tile as tile
from concourse import bacc
import concourse.mybir as mybir
from concourse import bass_utils

f32 = mybir.dt.float32
f32r = mybir.dt.float32r
bf16 = mybir.dt.bfloat16
AF = mybir.ActivationFunctionType

N_CORES = 8
HEADS = 8
D = 64            # head dim
B = 2             # batch
SEQ = 2048        # tokens per batch
DIM = 512         # model dim = HEADS * D
NTOK = B * SEQ    # 4096
EPS = 1e-4
SCALE = D ** -0.5  # 0.125

FT = DIM // 128   # 4 f-tiles of 128
CH1 = 512         # stage-1 token chunk
NCH1 = NTOK // CH1            # 8
ICH = 1024        # phase-2 i-chunk (exp batching)
NICH = SEQ // ICH             # 2 per batch
JT = SEQ // 128   # 16 j-tiles per batch
PCH = 512         # phase-3 chunk
NPCH = SEQ // PCH             # 4 per batch

_BUILD_CACHE = {}


def build(collective=True, num_devices=N_CORES, reps=1):
    key = (collective, num_devices, reps)
    if key in _BUILD_CACHE:
        return _BUILD_CACHE[key]
    nc = bacc.Bacc("TRN2", target_bir_lowering=False, debug=False,
                   num_devices=num_devices)
    xT = nc.dram_tensor("xT", [DIM, NTOK], f32, kind="ExternalInput").ap()
    wqk = nc.dram_tensor("wqk", [DIM, 128], f32, kind="ExternalInput").ap()
    wv = nc.dram_tensor("wv", [DIM, D], f32, kind="ExternalInput").ap()
    w2 = nc.dram_tensor("w2", [DIM, D], bf16, kind="ExternalInput").ap()
    ones2 = nc.dram_tensor("ones2", [128, 2], f32, kind="ExternalInput").ap()
    e2 = nc.dram_tensor("e2", [2, 128], f32, kind="ExternalInput").ap()
    ones1 = nc.dram_tensor("ones1", [1, D], f32, kind="ExternalInput").ap()
    outT = nc.dram_tensor("outT", [D, NTOK], f32, kind="ExternalOutput").ap()

    with tile.TileContext(nc) as tc:
        with (
            tc.tile_pool(name="persist", bufs=1) as pp,
            tc.tile_pool(name="sb", bufs=2) as sb,
            tc.tile_pool(name="ps", bufs=1, space="PSUM") as ps,
            tc.tile_pool(name="dram", bufs=1, space="DRAM") as dram,
            nc.allow_low_precision(reason="f32r matmuls; bf16 attn/out path"),
        ):
            # ---- persistent weights / constants ----
            wqk_sb = pp.tile([128, FT, 128], f32r)
            wv_sb = pp.tile([128, FT, D], f32r)
            w2_sb = pp.tile([128, FT, D], bf16)
            for t in range(FT):
                nc.sync.dma_start(wqk_sb[:, t, :],
                                  wqk[t * 128:(t + 1) * 128, :].bitcast(f32r))
                nc.sync.dma_start(wv_sb[:, t, :],
                                  wv[t * 128:(t + 1) * 128, :].bitcast(f32r))
                nc.sync.dma_start(w2_sb[:, t, :], w2[t * 128:(t + 1) * 128, :])
            o2_sb = pp.tile([128, 2], f32r)
            nc.sync.dma_start(o2_sb[:], ones2[:].bitcast(f32r))
            e2_sb = pp.tile([2, 128], f32r)
            nc.sync.dma_start(e2_sb[:], e2[:].bitcast(f32r))
            o1_sb = pp.tile([1, D], f32r)
            nc.sync.dma_start(o1_sb[:], ones1[:].bitcast(f32r))

            # ---- persistent activations ----
            qn_sb = pp.tile([D, NTOK], f32r)     # normalized qT
            kn_sb = pp.tile([D, NTOK], f32r)     # normalized kT
            qk_all = pp.tile([128, NTOK], f32r)  # raw [q;k]T
            vo_sb = pp.tile([128, NTOK // 128, D + 1], bf16)  # v | ones

            # ---- stage 1: qkv projections + rms normalization ----
            for rep in range(reps):
              for ci in range(NCH1):
                  cols = slice(ci * CH1, (ci + 1) * CH1)
                  xt_sb = sb.tile([128, FT, CH1], f32r, tag="xt")
                  for t in range(FT):
                      nc.sync.dma_start(
                          xt_sb[:, t, :],
                          xT[t * 128:(t + 1) * 128, cols].bitcast(f32r))

                  # qkT chunk: [q;k] x-stream, W stationary
                  qk_ps = ps.tile([128, CH1], f32, tag="small", bufs=4)
                  for t in range(FT):
                      nc.tensor.matmul(qk_ps[:], wqk_sb[:, t, :], xt_sb[:, t, :],
                                       start=(t == 0), stop=(t == FT - 1))
                  # v chunk: [j, d], xT stationary
                  for js in range(CH1 // 128):
                      v_ps = ps.tile([128, D], f32, tag="small", bufs=4)
                      for t in range(FT):
                          nc.tensor.matmul(
                              v_ps[:],
                              xt_sb[:, t, js * 128:(js + 1) * 128],
                              wv_sb[:, t, :],
                              start=(t == 0), stop=(t == FT - 1))
                      jt = ci * (CH1 // 128) + js
                      nc.vector.tensor_copy(vo_sb[:, jt, 0:D], v_ps[:])
                      nc.gpsimd.memset(vo_sb[:, jt, D:D + 1], 1.0)

                  # raw qk to sbuf (ACT; DVE is busier)
                  nc.scalar.activation(qk_all[:, cols], qk_ps[:], AF.Copy)
                  # sq = qk^2 (DVE, from the sbuf copy)
                  sq_sb = sb.tile([128, CH1], f32r, tag="sq")
                  nc.vector.tensor_mul(sq_sb[:], qk_all[:, cols],
                                       qk_all[:, cols])
                  # st[2, CH1] = column sums of q-sq and k-sq
                  st_ps = ps.tile([2, CH1], f32, tag="small", bufs=4)
                  nc.tensor.matmul(st_ps[:], o2_sb[:], sq_sb[:],
                                   start=True, stop=True)
                  # r = 1/(sqrt(st/64) + eps)
                  rt_sb = sb.tile([2, CH1], f32, tag="rt")
                  nc.scalar.activation(rt_sb[:], st_ps[:], AF.Sqrt,
                                       scale=1.0 / D)
                  re_sb = sb.tile([2, CH1], f32, tag="re")
                  nc.vector.tensor_scalar_add(re_sb[:], rt_sb[:], EPS)
                  rc_sb = sb.tile([2, CH1], f32r, tag="rc")
                  nc.vector.reciprocal(rc_sb[:], re_sb[:])
                  # R[128, CH1] = outer(e2, r): row broadcast of scales
                  r_ps = ps.tile([128, CH1], f32, tag="small", bufs=4)
                  nc.tensor.matmul(r_ps[:], e2_sb[:], rc_sb[:],
                                   start=True, stop=True)
                  rb_sb = sb.tile([128, CH1], f32r, tag="rb")
                  nc.scalar.activation(rb_sb[:], r_ps[:], AF.Copy)
                  # apply
                  nc.vector.tensor_mul(qn_sb[:, cols], qk_all[0:D, cols],
                                       rb_sb[0:D, :])
                  nc.vector.tensor_mul(kn_sb[:, cols], qk_all[D:128, cols],
                                       rb_sb[D:128, :])

              # ---- per-batch: attention + allgather + out-proj ----
              cc_outs = []
              for b in range(B):
                  cc_in = dram.tile([D, SEQ], bf16, name=f"cc_in{b}")
                  cc_out = dram.tile([DIM, SEQ], bf16, addr_space="Shared",
                                     name=f"cc_out{b}")
                  cc_outs.append(cc_out)
                  for ic in range(NICH):
                      i0 = b * SEQ + ic * ICH
                      expT = sb.tile([128, JT, ICH], bf16, tag="expT")
                      for jt in range(JT):
                          j0 = b * SEQ + jt * 128
                          sim_ps = ps.tile([128, ICH], f32, tag="big", bufs=2)
                          for h in range(ICH // 512):
                              nc.tensor.matmul(
                                  sim_ps[:, h * 512:(h + 1) * 512],
                                  kn_sb[:, j0:j0 + 128],
                                  qn_sb[:, i0 + h * 512:i0 + (h + 1) * 512],
                                  start=True, stop=True)
                          nc.scalar.activation(expT[:, jt, :], sim_ps[:],
                                               AF.Exp, scale=SCALE)
                      for h in range(ICH // 512):
                          av_ps = ps.tile([D + 1, 512], f32, tag="small", bufs=4)
                          for jt in range(JT):
                              nc.tensor.matmul(
                                  av_ps[:],
                                  vo_sb[:, b * JT + jt, :],
                                  expT[:, jt, h * 512:(h + 1) * 512],
                                  start=(jt == 0), stop=(jt == JT - 1))
                          # normalize by sumexp (row D) and emit bf16
                          rse_sb = sb.tile([1, 512], f32r, tag="rse")
                          nc.vector.reciprocal(rse_sb[:],
                                               av_ps[D:D + 1, :].bitcast(f32r))
                          r2_ps = ps.tile([D, 512], f32, tag="small", bufs=4)
                          nc.tensor.matmul(r2_ps[:], o1_sb[:], rse_sb[:],
                                           start=True, stop=True)
                          r2_sb = sb.tile([D, 512], f32, tag="r2")
                          nc.scalar.activation(r2_sb[:], r2_ps[:], AF.Copy)
                          oc_sb = sb.tile([D, 512], bf16, tag="oc")
                          nc.vector.tensor_mul(oc_sb[:], av_ps[0:D, :], r2_sb[:])
                          nc.sync.dma_start(
                              cc_in[:, ic * ICH + h * 512:
                                    ic * ICH + (h + 1) * 512], oc_sb[:])
                  if collective:
                      nc.gpsimd.collective_compute(
                          "AllGather", mybir.AluOpType.bypass,
                          replica_groups=[list(range(num_devices))],
                          ins=[cc_in[:]], outs=[cc_out[:]])
                  else:
                      # timing-only stand-in: keep the DRAM write traffic
                      nc.sync.dma_start(cc_out[0:D, :], cc_in[:])

              for b in range(B):
                  cc_out = cc_outs[b]
                  for pc in range(NPCH):
                      cols = slice(pc * PCH, (pc + 1) * PCH)
                      ag_sb = sb.tile([128, FT, PCH], bf16, tag="ag")
                      for t in range(FT):
                          nc.sync.dma_start(ag_sb[:, t, :],
                                            cc_out[t * 128:(t + 1) * 128, cols])
                      fp_ps = ps.tile([D, PCH], f32, tag="small", bufs=4)
                      for t in range(FT):
                          nc.tensor.matmul(fp_ps[:], w2_sb[:, t, :],
                                           ag_sb[:, t, :],
                                           start=(t == 0), stop=(t == FT - 1))
                      fo_sb = sb.tile([D, PCH], f32, tag="fo")
                      nc.vector.tensor_copy(fo_sb[:], fp_ps[:])
                      nc.sync.dma_start(
                          outT[:, b * SEQ + pc * PCH:b * SEQ + (pc + 1) * PCH],
                          fo_sb[:])
    nc.compile()
    _BUILD_CACHE[key] = nc
    return nc


def make_in_maps(x, Wq, Wkv, Wout):
    xT = np.ascontiguousarray(x.reshape(NTOK, DIM).T).astype(np.float32)
    ones2 = np.zeros((128, 2), np.float32)
    ones2[0:D, 0] = 1.0
    ones2[D:128, 1] = 1.0
    e2 = np.ascontiguousarray(ones2.T)
    ones1 = np.ones((1, D), np.float32)
    in_maps = []
    for c in range(N_CORES):
        rows = slice(c * D, (c + 1) * D)
        wqk = np.ascontiguousarray(
            np.concatenate([Wq[rows, :].T, Wkv[rows, :].T], axis=1))
        wv = np.ascontiguousarray(Wkv[DIM + c * D:DIM + (c + 1) * D, :].T)
        w2 = np.ascontiguousarray(Wout[rows, :].T).astype(ml_dtypes.bfloat16)
        in_maps.append({
            "xT": xT, "wqk": wqk.astype(np.float32),
            "wv": wv.astype(np.float32), "w2": w2,
            "ones2": ones2, "e2": e2, "ones1": ones1,
        })
    return in_maps


def kernel(x, Wq, Wkv, Wout, _trace=False):
    nc = build()
    in_maps = make_in_maps(np.asarray(x), np.asarray(Wq), np.asarray(Wkv),
                           np.asarray(Wout))
    res = bass_utils.run_bass_kernel_spmd(
        nc, in_maps, core_ids=list(range(N_CORES)), trace=_trace)
    out = np.empty((NTOK, DIM), np.float32)
    for c in range(N_CORES):
        out[:, c * D:(c + 1) * D] = res.results[c]["outT"].T
    full = out.reshape(B, SEQ, DIM)
    if _trace:
        return full, res
    return full



# revision 31
# speedup vs baseline: 1.1827x; 1.1827x over previous
"""CosineAttention on 8 TRN2 NeuronCores — v2.

Sharding: core c owns batch g=c//4 and heads {2r, 2r+1} (r=c%4).
The two heads are stacked on the 128-partition axis everywhere:
  - q/k projections produce [128, n] tiles (rows 0-63 head A, 64-127 head B)
  - RMS scale r = exp(-0.5*ln(mean sq)) on ACT (single ln/exp table set for
    the whole kernel -> no activation-table switches)
  - sim: K=64 matmuls 2-way row-tiled via tile_position (0,0)/(64,0) so both
    heads' sims run concurrently in the PE array
  - exp at [128, 1024] granularity (both heads per instr) straight from PSUM
  - attn@v with a ones column appended to v (softmax denominator = row 64)
  - 1/sumexp via DVE reciprocal_approx_fast (no ACT table pressure)
  - per-512-token-chunk AllGather over the 4 cores of the batch group,
    overlapped with attention of later chunks; column-parallel out-proj.
All PE operands bf16 except small f32r helper matmuls (N=512 -> full rate).
"""

import numpy as np
import ml_dtypes

import concourse.bass as bass
import concourse.tile as tile
from concourse import bacc
import concourse.mybir as mybir
from concourse import bass_utils

f32 = mybir.dt.float32
f32r = mybir.dt.float32r
bf16 = mybir.dt.bfloat16
AF = mybir.ActivationFunctionType

N_CORES = 8
HEADS = 8
D = 64            # head dim
B = 2             # batch
SEQ = 2048        # tokens per batch (= tokens per core)
DIM = 512         # model dim
SCALE = D ** -0.5  # 0.125

FT = DIM // 128   # 4 k-tiles of 128
CH = 512          # stage-1 token chunk
NCH = SEQ // CH               # 4
JT = SEQ // 128   # 16 j-tiles
IH = 512          # attention i-half chunk
NIH = SEQ // IH               # 4 (i chunks of 512, also collective chunks)

_BUILD_CACHE = {}


def build(collective=True, num_devices=N_CORES, dbg=False):
    key = (collective, num_devices, dbg)
    if key in _BUILD_CACHE:
        return _BUILD_CACHE[key]
    nc = bacc.Bacc("TRN2", target_bir_lowering=False, debug=False,
                   num_devices=num_devices)
    xT = nc.dram_tensor("xT", [DIM, SEQ], bf16, kind="ExternalInput").ap()
    wq = nc.dram_tensor("wq", [DIM, 128], bf16, kind="ExternalInput").ap()
    wk = nc.dram_tensor("wk", [DIM, 128], bf16, kind="ExternalInput").ap()
    wv = nc.dram_tensor("wv", [DIM, 128], bf16, kind="ExternalInput").ap()
    w2 = nc.dram_tensor("w2", [DIM, 128], bf16, kind="ExternalInput").ap()
    o2 = nc.dram_tensor("o2", [128, 2], f32, kind="ExternalInput").ap()
    o1 = nc.dram_tensor("o1", [1, 128], f32, kind="ExternalInput").ap()
    e2 = nc.dram_tensor("e2", [34, 128], f32, kind="ExternalInput").ap()
    outT = nc.dram_tensor("outT", [128, SEQ], f32, kind="ExternalOutput").ap()
    if dbg:
        qnD = nc.dram_tensor("qnD", [128, SEQ], f32, kind="ExternalOutput").ap()
        knD = nc.dram_tensor("knD", [128, SEQ], f32, kind="ExternalOutput").ap()
        voD = nc.dram_tensor("voD", [128, JT * 130], f32,
                             kind="ExternalOutput").ap()
        exD = nc.dram_tensor("exD", [128, 2 * IH], f32,
                             kind="ExternalOutput").ap()
        avD = nc.dram_tensor("avD", [128, 2 * IH], f32,
                             kind="ExternalOutput").ap()
        ocD = nc.dram_tensor("ocD", [128, IH], f32, kind="ExternalOutput").ap()
        rsD = nc.dram_tensor("rsD", [1, 2 * IH], f32, kind="ExternalOutput").ap()
        r2D = nc.dram_tensor("r2D", [128, IH], f32, kind="ExternalOutput").ap()

    with tile.TileContext(nc) as tc:
        with (
            tc.tile_pool(name="persist", bufs=1) as pp,
            tc.tile_pool(name="sb", bufs=2) as sb,
            tc.tile_pool(name="ps", bufs=1, space="PSUM") as ps,
            tc.tile_pool(name="dram", bufs=1, space="DRAM") as dram,
            nc.allow_low_precision(reason="bf16 matmuls; f32r helpers"),
        ):
            # ---- persistent weights / constants ----
            wq_sb = pp.tile([128, FT, 128], bf16)
            wk_sb = pp.tile([128, FT, 128], bf16)
            wv_sb = pp.tile([128, FT, 128], bf16)
            w2_sb = pp.tile([128, FT, 128], bf16)
            for w_sb, w_dr in ((wq_sb, wq), (wk_sb, wk), (wv_sb, wv),
                               (w2_sb, w2)):
                nc.sync.dma_start(
                    w_sb[:], w_dr.rearrange("(t p) m -> p t m", p=128))
            o2_sb = pp.tile([128, 2], f32r)
            nc.sync.dma_start(o2_sb[:], o2[:].bitcast(f32r))
            e2_sb = pp.tile([34, 128], f32r)
            nc.sync.dma_start(e2_sb[:], e2[:].bitcast(f32r))
            o1f_sb = pp.tile([1, 128], f32)
            nc.sync.dma_start(o1f_sb[:], o1[:])

            # ---- persistent activations ----
            qn_sb = pp.tile([128, SEQ], bf16)    # rows 0-63 hA, 64-127 hB
            kn_sb = pp.tile([128, SEQ], bf16)
            vo_sb = pp.tile([128, JT, 130], bf16)  # [v_hA|1|v_hB|1] per j-tile
            nc.gpsimd.memset(vo_sb[:], 1.0)

            # ---- stage 1: projections + rms normalization ----
            for c in range(NCH):
                cols = slice(c * CH, (c + 1) * CH)
                xt = sb.tile([128, FT, CH], bf16, tag="xt", bufs=3)
                nc.sync.dma_start(
                    xt[:], xT[:, cols].rearrange("(t p) n -> p t n", p=128))

                qk_ps = ps.tile([128, 2, CH], f32, tag="sim", bufs=2)
                for t in range(FT):
                    nc.tensor.matmul(qk_ps[:, 0, :], wq_sb[:, t, :],
                                     xt[:, t, :],
                                     start=(t == 0), stop=(t == FT - 1))
                for t in range(FT):
                    nc.tensor.matmul(qk_ps[:, 1, :], wk_sb[:, t, :],
                                     xt[:, t, :],
                                     start=(t == 0), stop=(t == FT - 1))
                # v: [tok, 2*64] per 128-token tile, packed into one bank
                vb_ps = ps.tile([128, CH], f32, tag="acc", bufs=4)
                for js in range(CH // 128):
                    for t in range(FT):
                        nc.tensor.matmul(
                            vb_ps[:, js * 128:(js + 1) * 128],
                            xt[:, t, js * 128:(js + 1) * 128],
                            wv_sb[:, t, :],
                            start=(t == 0), stop=(t == FT - 1))
                # sq = q^2 / k^2 (ACT Square; same table set as ln/exp)
                sq = sb.tile([128, 2, CH], f32r, tag="sq")
                nc.scalar.activation(sq[:], qk_ps[:], AF.Square)
                # st_q/st_k: per-head column sums of squares (rows 0-1)
                st_q = ps.tile([128, CH], f32, tag="acc", bufs=4)
                st_k = ps.tile([128, CH], f32, tag="acc", bufs=4)
                nc.tensor.matmul(st_q[0:2, :], o2_sb[:], sq[:, 0, :],
                                 start=True, stop=True)
                nc.tensor.matmul(st_k[0:2, :], o2_sb[:], sq[:, 1, :],
                                 start=True, stop=True)
                # r = (st/64)^-0.5 via ln+exp (same ACT table set as attention)
                lt = sb.tile([2, 2, CH], f32, tag="lt")
                nc.scalar.activation(lt[:, 0, :], st_q[0:2, :], AF.Ln,
                                     scale=1.0 / D)
                nc.scalar.activation(lt[:, 1, :], st_k[0:2, :], AF.Ln,
                                     scale=1.0 / D)
                r4 = sb.tile([2, 2, CH], f32r, tag="r4")
                nc.scalar.activation(r4[:], lt[:], AF.Exp, scale=-0.5)
                # broadcast r over the 64 rows of each head (PE outer product)
                rb_ps = ps.tile([128, 2, CH], f32, tag="sim", bufs=2)
                nc.tensor.matmul(rb_ps[:, 0, :], e2_sb[0:2, :], r4[:, 0, :],
                                 start=True, stop=True)
                nc.tensor.matmul(rb_ps[:, 1, :], e2_sb[0:2, :], r4[:, 1, :],
                                 start=True, stop=True)
                # apply scales (rb via ACT copy to SBUF: PSUM allows only one
                # PSUM operand per DVE instruction)
                rb_sb = sb.tile([128, 2, CH], f32, tag="rb")
                nc.scalar.activation(rb_sb[:], rb_ps[:], AF.Copy)
                nc.vector.tensor_mul(qn_sb[:, cols], qk_ps[:, 0, :],
                                     rb_sb[:, 0, :])
                nc.vector.tensor_mul(kn_sb[:, cols], qk_ps[:, 1, :],
                                     rb_sb[:, 1, :])
                # v -> vo (cols 0-63 -> 0-63, 64-127 -> 65-128)
                for js in range(CH // 128):
                    jt = c * (CH // 128) + js
                    nc.vector.tensor_copy(
                        vo_sb[:, jt, 0:64],
                        vb_ps[:, js * 128:js * 128 + 64])
                    nc.vector.tensor_copy(
                        vo_sb[:, jt, 65:129],
                        vb_ps[:, js * 128 + 64:(js + 1) * 128])

            if dbg:
                for c in range(NCH):
                    cols = slice(c * CH, (c + 1) * CH)
                    for src, dst in ((qn_sb, qnD), (kn_sb, knD)):
                        dt_ = sb.tile([128, CH], f32, tag="dbg")
                        nc.vector.tensor_copy(dt_[:], src[:, cols])
                        nc.sync.dma_start(dst[:, cols], dt_[:])
                dt_ = sb.tile([128, JT, 130], f32, tag="dbgv")
                nc.vector.tensor_copy(dt_[:], vo_sb[:])
                nc.sync.dma_start(voD[:].rearrange("p (j c) -> p j c", j=JT),
                                  dt_[:])

            # ---- attention + chunked collective + out-proj ----
            for g in range(NIH):
                i0 = g * IH
                cc_in = dram.tile([128, IH], bf16, name=f"cc_in{g}")
                cc_out = dram.tile([DIM, IH], bf16, name=f"cc_out{g}")
                av_a = ps.tile([128, IH], f32, tag="acc", bufs=4)
                av_b = ps.tile([128, IH], f32, tag="acc", bufs=4)
                for jt in range(JT):
                    jc = slice(jt * 128, (jt + 1) * 128)
                    simp = ps.tile([128, 2, IH], f32, tag="sim", bufs=2)
                    nc.tensor.matmul(simp[:, 0, :], kn_sb[0:64, jc],
                                     qn_sb[0:64, i0:i0 + IH],
                                     start=True, stop=True,
                                     tile_position=(0, 0))
                    nc.tensor.matmul(simp[:, 1, :], kn_sb[64:128, jc],
                                     qn_sb[64:128, i0:i0 + IH],
                                     start=True, stop=True,
                                     tile_position=(64, 0))
                    expt = sb.tile([128, 2, IH], bf16, tag="expt", bufs=3)
                    nc.scalar.activation(expt[:], simp[:], AF.Exp, scale=SCALE)
                    nc.tensor.matmul(av_a[0:65, :], vo_sb[:, jt, 0:65],
                                     expt[:, 0, :],
                                     start=(jt == 0), stop=(jt == JT - 1),
                                     skip_group_check=True)
                    nc.tensor.matmul(av_b[0:65, :], vo_sb[:, jt, 65:130],
                                     expt[:, 1, :],
                                     start=(jt == 0), stop=(jt == JT - 1),
                                     skip_group_check=True)
                    if dbg and g == 0 and jt == 0:
                        dt_ = sb.tile([128, 2, IH], f32, tag="dbge")
                        nc.vector.tensor_copy(dt_[:], expt[:])
                        nc.sync.dma_start(
                            exD[:].rearrange("p (h i) -> p h i", h=2), dt_[:])
                # normalize by 1/sumexp (row 64) and emit bf16
                se_sb = sb.tile([1, 2, IH], f32, tag="se")
                nc.vector.tensor_copy(se_sb[0:1, 0, :], av_a[64:65, :])
                nc.vector.tensor_copy(se_sb[0:1, 1, :], av_b[64:65, :])
                rsec = sb.tile([1, 2, IH], f32, tag="rsec")
                nc.vector.reciprocal_approx_fast(rsec[0:1, 0, :],
                                                 se_sb[0:1, 0, :])
                nc.vector.reciprocal_approx_fast(rsec[0:1, 1, :],
                                                 se_sb[0:1, 1, :])
                r2_ps = ps.tile([128, IH], f32, tag="acc", bufs=4)
                nc.tensor.matmul(r2_ps[0:64, :], o1f_sb[0:1, 0:64],
                                 rsec[0:1, 0, :], start=True, stop=True)
                nc.tensor.matmul(r2_ps[64:128, :], o1f_sb[0:1, 64:128],
                                 rsec[0:1, 1, :], start=True, stop=True,
                                 tile_position=(0, 64))
                if dbg and g == 0:
                    dt_ = sb.tile([128, 2 * IH], f32, tag="dbga")
                    nc.vector.tensor_copy(dt_[:, 0:IH], av_a[:])
                    nc.vector.tensor_copy(dt_[:, IH:2 * IH], av_b[:])
                    nc.sync.dma_start(avD[:], dt_[:])
                r2_sb = sb.tile([128, IH], f32, tag="r2")
                nc.vector.tensor_copy(r2_sb[:], r2_ps[:])
                if dbg and g == 0:
                    nc.sync.dma_start(
                        rsD[:].rearrange("p (h i) -> p h i", h=2), rsec[:])
                    nc.sync.dma_start(r2D[:], r2_sb[:])
                occ = sb.tile([128, IH], bf16, tag="occ")
                nc.vector.tensor_mul(occ[0:64, :], av_a[0:64, :],
                                     r2_sb[0:64, :])
                nc.vector.tensor_mul(occ[64:128, :], av_b[0:64, :],
                                     r2_sb[64:128, :])
                if dbg and g == 0:
                    dt_ = sb.tile([128, IH], f32, tag="dbgo")
                    nc.vector.tensor_copy(dt_[:], occ[:])
                    nc.sync.dma_start(ocD[:], dt_[:])
                nc.sync.dma_start(cc_in[:], occ[:])
                if collective:
                    nc.gpsimd.collective_compute(
                        "AllGather", mybir.AluOpType.bypass,
                        replica_groups=[[0, 1, 2, 3], [4, 5, 6, 7]],
                        ins=[cc_in[:]], outs=[cc_out[:]])
                else:
                    nc.sync.dma_start(cc_out[0:128, :], cc_in[:])
                # column-parallel out-proj for this token chunk
                ag = sb.tile([128, FT, IH], bf16, tag="ag")
                nc.sync.dma_start(
                    ag[:], cc_out[:].rearrange("(t p) n -> p t n", p=128))
                op_ps = ps.tile([128, IH], f32, tag="acc", bufs=4)
                for t in range(FT):
                    nc.tensor.matmul(op_ps[:], w2_sb[:, t, :], ag[:, t, :],
                                     start=(t == 0), stop=(t == FT - 1))
                fo = sb.tile([128, IH], f32, tag="fo")
                nc.vector.tensor_copy(fo[:], op_ps[:])
                nc.sync.dma_start(outT[:, i0:i0 + IH], fo[:])
    nc.compile()
    _BUILD_CACHE[key] = nc
    return nc


def make_in_maps(x, Wq, Wkv, Wout):
    o2 = np.zeros((128, 2), np.float32)
    o2[0:64, 0] = 1.0
    o2[64:128, 1] = 1.0
    o1 = np.ones((1, 128), np.float32)
    e2 = np.zeros((34, 128), np.float32)
    e2[0, 0:64] = 1.0
    e2[1, 64:128] = 1.0
    e2[32, 0:64] = 1.0
    e2[33, 64:128] = 1.0
    bf = ml_dtypes.bfloat16
    in_maps = []
    for c in range(N_CORES):
        g, r = c // 4, c % 4
        hrows = slice(2 * r * D, (2 * r + 2) * D)
        xT = np.ascontiguousarray(x[g].T).astype(bf)
        wq = np.ascontiguousarray(Wq[hrows, :].T).astype(bf)
        wk = np.ascontiguousarray(Wkv[hrows, :].T).astype(bf)
        wv = np.ascontiguousarray(
            Wkv[DIM + 2 * r * D:DIM + (2 * r + 2) * D, :].T).astype(bf)
        w2 = np.ascontiguousarray(Wout[128 * r:128 * (r + 1), :].T).astype(bf)
        in_maps.append({
            "xT": xT, "wq": wq, "wk": wk, "wv": wv, "w2": w2,
            "o2": o2, "o1": o1, "e2": e2,
        })
    return in_maps


def kernel(x, Wq, Wkv, Wout, _trace=False):
    nc = build()
    in_maps = make_in_maps(np.asarray(x), np.asarray(Wq), np.asarray(Wkv),
                           np.asarray(Wout))
    res = bass_utils.run_bass_kernel_spmd(
        nc, in_maps, core_ids=list(range(N_CORES)), trace=_trace)
    full = np.empty((B, SEQ, DIM), np.float32)
    for c in range(N_CORES):
        g, r = c // 4, c % 4
        full[g, :, 128 * r:128 * (r + 1)] = res.results[c]["outT"].T
    if _trace:
        return full, res
    return full
